# revision 7
# baseline (speedup 1.0000x reference)
"""Trainium2 Bass kernel for nn_EstimateRebuildModule.

Reference computation:
    offsetmap = (Offset*4 + mean_posmap) -> [B,H,W,3], y-channel sign-flipped
    src/dst   = 68 keypoints gathered from offsetmap / Posmap_kpt
    per-sample Umeyama similarity (3x3 SVD) -> sR, t
    out       = offsetmap @ sR.T + t -> [B,3,H,W]

Strategy (pure data parallel, batch sharded across 8 NeuronCores):
  * The per-sample transform only needs 68 pixels of each input to estimate
    sR/t; that tiny reduction (64 3x3 SVDs) runs on the host in f64.
  * The memory-bound bulk - reading Offset (48MB), applying the per-sample
    3x3 affine map to every pixel, writing the output (48MB) - runs on
    device.  Posmap_kpt's 48MB is never touched by the device (only its 68
    gathered pixels matter), so per-core traffic is ~12.9MB.
  * On each core the whole pixel transform is one stationary matmul:
    K=109 rows = 96 Offset rows (8 samples x 3 ch x 4 pixel-groups,
    block-diagonal weights) + 12 mean_posmap rows + 1 ones row (bias t),
    M=96 output rows, streamed over 16384 pixel columns.
  * MODE fp16x3: operands are split on the host into fp16 hi+lo halves
    (same DMA bytes as fp32) and the product is computed as
    Whi@xhi + Whi@xlo + Wlo@xhi, three 1-cycle/column fp16 matmuls -
    vs fp32's single 4-cycle/column pass.  absmax error matches fp32.
"""

import os
import numpy as np

B, C, H, W = 64, 3, 256, 256
N_CORES = 8
BPC = B // N_CORES          # samples per core
PIX = H * W                 # 65536
NG = 4                      # pixel groups packed into the partition dim
QP = PIX // NG              # 16384 pixel columns per group
KO = NG * BPC * C           # 96 Offset rows
KM = NG * C + 1             # 12 mean rows + 1 ones row
K = KO + KM                 # 109
M = NG * BPC * C            # 96 output rows
TILE = 2048                 # pixel columns per tile
NTILES = QP // TILE         # 8
REVERT = np.array([1.0, -1.0, 1.0], np.float32)
OFFSET_SCALE = 4.0

MODE = os.environ.get("KERNEL_MODE", "fp16x3")   # "fp32" | "fp16x3"
LDW_OPT = os.environ.get("KERNEL_LDW_OPT", "1") == "1"

_STATE = {}
LAST_RESULTS = None


def _patch_ldw_opt():
    """Enable walrus's redundant-LDWEIGHTS elision (--enable-ldw-opt).
    concourse hardcodes it off; with a stationary weight reused across many
    matmuls the reloads are pure overhead (~190ns each on PE)."""
    if _STATE.get("ldw_patched") or not LDW_OPT:
        return
    from concourse import bass_utils
    orig = bass_utils.run_command

    def run_command_ldw(argv, **kw):
        argv = ["--enable-ldw-opt=true" if a == "--enable-ldw-opt=false" else a
                for a in argv]
        return orig(argv, **kw)

    bass_utils.run_command = run_command_ldw
    _STATE["ldw_patched"] = True


def _build_fp32():
    import concourse.bacc as bacc
    import concourse.tile as tile
    from concourse import mybir
    from contextlib import ExitStack

    f32 = mybir.dt.float32
    nc = bacc.Bacc("TRN2", target_bir_lowering=False, debug=False,
                   num_devices=N_CORES)
    offs = nc.dram_tensor("offs", [KO, QP], f32, kind="ExternalInput").ap()
    meanr = nc.dram_tensor("meanr", [KM, QP], f32, kind="ExternalInput").ap()
    wmat = nc.dram_tensor("wmat", [K, M], f32, kind="ExternalInput").ap()
    out = nc.dram_tensor("out", [M, QP], f32, kind="ExternalOutput").ap()

    with tile.TileContext(nc) as tc, ExitStack() as ctx:
        consts = ctx.enter_context(tc.tile_pool(name="consts", bufs=1))
        movp = ctx.enter_context(tc.tile_pool(name="mov", bufs=3))
        outp = ctx.enter_context(tc.tile_pool(name="outt", bufs=3))
        psump = ctx.enter_context(tc.tile_pool(name="ps", bufs=2, space="PSUM"))

        wt = consts.tile([K, M], f32)
        nc.sync.dma_start(wt[:], wmat[:])

        for i in range(NTILES):
            sl_i = slice(i * TILE, (i + 1) * TILE)
            mov = movp.tile([K, TILE], f32)
            nc.sync.dma_start(mov[0:KO, :], offs[:, sl_i])
            nc.sync.dma_start(mov[KO:K, :], meanr[:, sl_i])
            ps = psump.tile([M, TILE], f32)
            for j in range(TILE // 512):
                sl_j = slice(j * 512, (j + 1) * 512)
                nc.tensor.matmul(ps[:, sl_j], wt[:], mov[:, sl_j],
                                 start=True, stop=True)
            ot = outp.tile([M, TILE], f32)
            if i % 2 == 0:
                nc.scalar.copy(ot[:], ps[:])
            else:
                nc.vector.tensor_copy(ot[:], ps[:])
            nc.scalar.dma_start(out[:, sl_i], ot[:])

    nc.compile()
    return nc


def _build_fp16x3():
    import concourse.bacc as bacc
    import concourse.tile as tile
    from concourse import mybir
    from contextlib import ExitStack

    f32 = mybir.dt.float32
    f16 = mybir.dt.float16
    nc = bacc.Bacc("TRN2", target_bir_lowering=False, debug=False,
                   num_devices=N_CORES)
    xhi = nc.dram_tensor("xhi", [K, QP], f16, kind="ExternalInput").ap()
    xlo = nc.dram_tensor("xlo", [K, QP], f16, kind="ExternalInput").ap()
    whi = nc.dram_tensor("whi", [K, M], f16, kind="ExternalInput").ap()
    wlo = nc.dram_tensor("wlo", [K, M], f16, kind="ExternalInput").ap()
    out = nc.dram_tensor("out", [M, QP], f32, kind="ExternalOutput").ap()

    # DMA tiles are 4096 cols so each fp16 partition-row is 8KB: the HWDGE
    # only splits a transfer across the 16 SDMA engines at >=8KB rows
    # (smaller rows aggregate into engine-atomic packets -> serial DMA).
    DTILE = 4096
    NDT = QP // DTILE              # 4 DMA tiles
    SUB = DTILE // TILE            # 2 psum sub-tiles per DMA tile

    with tile.TileContext(nc) as tc, ExitStack() as ctx:
        consts = ctx.enter_context(tc.tile_pool(name="consts", bufs=1))
        movh = ctx.enter_context(tc.tile_pool(name="movh", bufs=3))
        movl = ctx.enter_context(tc.tile_pool(name="movl", bufs=3))
        outp = ctx.enter_context(tc.tile_pool(name="outt", bufs=3))
        psump = ctx.enter_context(tc.tile_pool(name="ps", bufs=2, space="PSUM"))

        # weights go on the scalar HWDGE queue - the input queue (sync) must
        # start streaming xhi immediately
        whi_t = consts.tile([K, M], f16)
        nc.scalar.dma_start(whi_t[:], whi[:])
        wlo_t = consts.tile([K, M], f16)
        nc.scalar.dma_start(wlo_t[:], wlo[:])

        NJ = TILE // 512
        for it in range(NDT):
            sl_t = slice(it * DTILE, (it + 1) * DTILE)
            mh = movh.tile([K, DTILE], f16)
            nc.sync.dma_start(mh[:], xhi[:, sl_t])
            ml = movl.tile([K, DTILE], f16)
            nc.sync.dma_start(ml[:], xlo[:, sl_t])
            for s in range(SUB):
                i = it * SUB + s
                sl_i = slice(i * TILE, (i + 1) * TILE)
                ps = psump.tile([M, TILE], f32)
                # pass-major order: consecutive matmuls share the stationary
                # operand so (with ldw-opt) the weight reload is elided
                for j in range(NJ):
                    sl_p = slice(j * 512, (j + 1) * 512)
                    sl_j = slice(s * TILE + j * 512, s * TILE + (j + 1) * 512)
                    nc.tensor.matmul(ps[:, sl_p], whi_t[:], mh[:, sl_j],
                                     start=True, stop=False)
                for j in range(NJ):
                    sl_p = slice(j * 512, (j + 1) * 512)
                    sl_j = slice(s * TILE + j * 512, s * TILE + (j + 1) * 512)
                    nc.tensor.matmul(ps[:, sl_p], whi_t[:], ml[:, sl_j],
                                     start=False, stop=False)
                for j in range(NJ):
                    sl_p = slice(j * 512, (j + 1) * 512)
                    sl_j = slice(s * TILE + j * 512, s * TILE + (j + 1) * 512)
                    nc.tensor.matmul(ps[:, sl_p], wlo_t[:], mh[:, sl_j],
                                     start=False, stop=True)
                ot = outp.tile([M, TILE], f32)
                if i % 2 == 0:
                    nc.scalar.copy(ot[:], ps[:])
                else:
                    nc.vector.tensor_copy(ot[:], ps[:])
                nc.scalar.dma_start(out[:, sl_i], ot[:])

    nc.compile()
    return nc


def _get_program():
    if "nc" not in _STATE:
        _patch_ldw_opt()
        nc = _build_fp16x3() if MODE == "fp16x3" else _build_fp32()
        # The neuron compile cache keys on the HLO wrapper, which does NOT
        # include the embedded BIR - two different bass programs with the
        # same I/O signature would collide.  Key the cache dir on the BIR.
        import hashlib, tempfile
        digest = hashlib.sha256(nc.to_json_bytes()).hexdigest()[:16]
        os.environ["NEURON_COMPILE_CACHE_URL"] = os.path.join(
            tempfile.gettempdir(), f"neuron-cache-{digest}")
        _STATE["nc"] = nc
    return _STATE["nc"]


def _umeyama_params(Offset, Posmap_kpt, mean_posmap, uv_kpt):
    """Host-side: gather 68 keypoints, per-sample Umeyama in f64.
    Returns G[b,d,c] = revert[c] * sR[b,d,c] and t[b,d]."""
    h_idx = uv_kpt[:, 0].astype(np.int64)
    w_idx = uv_kpt[:, 1].astype(np.int64)
    # src[b,k,c] = revert[c] * (4*Offset[b,c,h,w] + mean[c,h,w])
    src = ((Offset[:, :, h_idx, w_idx] * OFFSET_SCALE
            + mean_posmap[:, h_idx, w_idx][None])
           * REVERT[None, :, None]).transpose(0, 2, 1).astype(np.float64)
    dst = Posmap_kpt[:, :, h_idx, w_idx].transpose(0, 2, 1).astype(np.float64)

    n = src.shape[1]
    mu_s = src.mean(axis=1)                      # [B,3]
    mu_d = dst.mean(axis=1)
    sd = src - mu_s[:, None, :]
    dd = dst - mu_d[:, None, :]
    A = np.einsum('bkd,bkc->bdc', dd, sd) / n    # [B,3,3] cross-covariance
    det = np.linalg.det(A)
    dvec = np.ones((B, 3))
    dvec[det < 0, 2] = -1.0
    U, S, Vt = np.linalg.svd(A)
    R = np.einsum('bik,bk,bkj->bij', U, dvec, Vt)
    var_s = (sd * sd).sum(axis=(1, 2)) / n
    scale = np.einsum('bk,bk->b', S, dvec) / var_s
    sR = scale[:, None, None] * R                # [B,3,3]
    t = mu_d - np.einsum('bij,bj->bi', sR, mu_s)
    G = sR * REVERT[None, None, :].astype(np.float64)   # fold revert into c
    return G, t


def _weights_for_core(G, t, core):
    """W[k, m]: m = g*24 + b*3 + d.  Rows: 96 Offset (block-diag per g,b),
    12 mean (block-diag per g, shared over b), 1 ones row (bias t)."""
    Wm = np.zeros((K, M), np.float32)
    Gc = G[core * BPC:(core + 1) * BPC]          # [8,3(d),3(c)]
    tc_ = t[core * BPC:(core + 1) * BPC]         # [8,3]
    for g in range(NG):
        for b in range(BPC):
            for c in range(C):
                for d in range(C):
                    m = g * 24 + b * 3 + d
                    Wm[g * 24 + b * 3 + c, m] = OFFSET_SCALE * Gc[b, d, c]
                    Wm[KO + g * 3 + c, m] = Gc[b, d, c]
                    Wm[KO + 12, m] = tc_[b, d]
    return Wm


def _split16(a):
    hi = a.astype(np.float16)
    lo = (a.astype(np.float32) - hi.astype(np.float32)).astype(np.float16)
    return np.ascontiguousarray(hi), np.ascontiguousarray(lo)


def kernel(Offset, Posmap_kpt, mean_posmap, uv_kpt):
    global LAST_RESULTS
    from concourse import bass_utils

    Offset = np.ascontiguousarray(np.asarray(Offset, dtype=np.float32))
    Posmap_kpt = np.asarray(Posmap_kpt, dtype=np.float32)
    mean_posmap = np.ascontiguousarray(np.asarray(mean_posmap, dtype=np.float32))
    uv = np.asarray(uv_kpt)

    G, t = _umeyama_params(Offset, Posmap_kpt, mean_posmap, uv)

    # [12, QP]: row g*3+c = mean[c, pixel-quarter g]; + ones row for bias
    meanr = np.ascontiguousarray(
        mean_posmap.reshape(C, NG, QP).transpose(1, 0, 2).reshape(NG * C, QP))
    meanr13 = np.concatenate([meanr, np.ones((1, QP), np.float32)], axis=0)

    in_maps = []
    for core in range(N_CORES):
        osh = Offset[core * BPC:(core + 1) * BPC]                 # [8,3,256,256]
        offs = np.ascontiguousarray(
            osh.reshape(BPC * C, NG, QP).transpose(1, 0, 2).reshape(KO, QP))
        Wm = _weights_for_core(G, t, core)
        if MODE == "fp16x3":
            x = np.concatenate([offs, meanr13], axis=0)           # [109, QP]
            xh, xl = _split16(x)
            wh, wl = _split16(Wm)
            in_maps.append({"xhi": xh, "xlo": xl, "whi": wh, "wlo": wl})
        else:
            in_maps.append({"offs": offs, "meanr": meanr13, "wmat": Wm})

    nc = _get_program()
    res = bass_utils.run_bass_kernel_spmd(nc, in_maps, core_ids=list(range(N_CORES)))
    LAST_RESULTS = res

    parts = []
    for core in range(N_CORES):
        o = res.results[core]["out"]                              # [96, QP]
        parts.append(o.reshape(NG, BPC * C, QP).transpose(1, 0, 2)
                      .reshape(BPC, C, H, W))
    return np.ascontiguousarray(np.concatenate(parts, axis=0)).astype(np.float32)


# revision 8
# speedup vs baseline: 4.5601x; 4.5601x over previous
"""Trainium2 Bass kernel for nn_EstimateRebuildModule.

Reference computation:
    offsetmap = (Offset*4 + mean_posmap) -> [B,H,W,3], y-channel sign-flipped
    src/dst   = 68 keypoints gathered from offsetmap / Posmap_kpt
    per-sample Umeyama similarity (3x3 SVD) -> sR, t
    out       = offsetmap @ sR.T + t -> [B,3,H,W]

Strategy (pure data parallel, batch sharded across 8 NeuronCores):
  * The per-sample transform only needs 68 pixels of each input to estimate
    sR/t; that tiny reduction (64 3x3 SVDs) runs on the host in f64.
  * The memory-bound bulk - reading Offset (48MB), applying the per-sample
    3x3 affine map to every pixel, writing the output (48MB) - runs on
    device.  Posmap_kpt's 48MB is never touched by the device (only its 68
    gathered pixels matter), so per-core traffic is ~12.9MB.
  * On each core the whole pixel transform is one stationary matmul:
    K=109 rows = 96 Offset rows (8 samples x 3 ch x 4 pixel-groups,
    block-diagonal weights) + 12 mean_posmap rows + 1 ones row (bias t),
    M=96 output rows, streamed over 16384 pixel columns.
  * MODE fp16x3: operands are split on the host into fp16 hi+lo halves
    (same DMA bytes as fp32) and the product is computed as
    Whi@xhi + Whi@xlo + Wlo@xhi, three 1-cycle/column fp16 matmuls -
    vs fp32's single 4-cycle/column pass.  absmax error matches fp32.
"""

import os
import numpy as np

B, C, H, W = 64, 3, 256, 256
N_CORES = 8
BPC = B // N_CORES          # samples per core
PIX = H * W                 # 65536
NG = 4                      # pixel groups packed into the partition dim
QP = PIX // NG              # 16384 pixel columns per group
KO = NG * BPC * C           # 96 Offset rows
KM = NG * C + 1             # 12 mean rows + 1 ones row
K = KO + KM                 # 109
M = NG * BPC * C            # 96 output rows
TILE = 2048                 # pixel columns per tile
NTILES = QP // TILE         # 8
REVERT = np.array([1.0, -1.0, 1.0], np.float32)
OFFSET_SCALE = 4.0

MODE = os.environ.get("KERNEL_MODE", "fp16x3")   # "fp32" | "fp16x3"
LDW_OPT = os.environ.get("KERNEL_LDW_OPT", "1") == "1"

_STATE = {}
LAST_RESULTS = None


def _patch_ldw_opt():
    """Enable walrus's redundant-LDWEIGHTS elision (--enable-ldw-opt).
    concourse hardcodes it off; with a stationary weight reused across many
    matmuls the reloads are pure overhead (~190ns each on PE)."""
    if _STATE.get("ldw_patched") or not LDW_OPT:
        return
    from concourse import bass_utils
    orig = bass_utils.run_command

    def run_command_ldw(argv, **kw):
        argv = ["--enable-ldw-opt=true" if a == "--enable-ldw-opt=false" else a
                for a in argv]
        return orig(argv, **kw)

    bass_utils.run_command = run_command_ldw
    _STATE["ldw_patched"] = True


def _build_fp32():
    import concourse.bacc as bacc
    import concourse.tile as tile
    from concourse import mybir
    from contextlib import ExitStack

    f32 = mybir.dt.float32
    nc = bacc.Bacc("TRN2", target_bir_lowering=False, debug=False,
                   num_devices=N_CORES)
    offs = nc.dram_tensor("offs", [KO, QP], f32, kind="ExternalInput").ap()
    meanr = nc.dram_tensor("meanr", [KM, QP], f32, kind="ExternalInput").ap()
    wmat = nc.dram_tensor("wmat", [K, M], f32, kind="ExternalInput").ap()
    out = nc.dram_tensor("out", [M, QP], f32, kind="ExternalOutput").ap()

    with tile.TileContext(nc) as tc, ExitStack() as ctx:
        consts = ctx.enter_context(tc.tile_pool(name="consts", bufs=1))
        movp = ctx.enter_context(tc.tile_pool(name="mov", bufs=3))
        outp = ctx.enter_context(tc.tile_pool(name="outt", bufs=3))
        psump = ctx.enter_context(tc.tile_pool(name="ps", bufs=2, space="PSUM"))

        wt = consts.tile([K, M], f32)
        nc.sync.dma_start(wt[:], wmat[:])

        for i in range(NTILES):
            sl_i = slice(i * TILE, (i + 1) * TILE)
            mov = movp.tile([K, TILE], f32)
            nc.sync.dma_start(mov[0:KO, :], offs[:, sl_i])
            nc.sync.dma_start(mov[KO:K, :], meanr[:, sl_i])
            ps = psump.tile([M, TILE], f32)
            for j in range(TILE // 512):
                sl_j = slice(j * 512, (j + 1) * 512)
                nc.tensor.matmul(ps[:, sl_j], wt[:], mov[:, sl_j],
                                 start=True, stop=True)
            ot = outp.tile([M, TILE], f32)
            if i % 2 == 0:
                nc.scalar.copy(ot[:], ps[:])
            else:
                nc.vector.tensor_copy(ot[:], ps[:])
            nc.scalar.dma_start(out[:, sl_i], ot[:])

    nc.compile()
    return nc


def _build_fp16x3():
    import concourse.bacc as bacc
    import concourse.tile as tile
    from concourse import mybir
    from contextlib import ExitStack

    f32 = mybir.dt.float32
    f16 = mybir.dt.float16
    nc = bacc.Bacc("TRN2", target_bir_lowering=False, debug=False,
                   num_devices=N_CORES)
    xhi = nc.dram_tensor("xhi", [K, QP], f16, kind="ExternalInput").ap()
    xlo = nc.dram_tensor("xlo", [K, QP], f16, kind="ExternalInput").ap()
    whi = nc.dram_tensor("whi", [K, M], f16, kind="ExternalInput").ap()
    wlo = nc.dram_tensor("wlo", [K, M], f16, kind="ExternalInput").ap()
    out = nc.dram_tensor("out", [M, QP], f32, kind="ExternalOutput").ap()

    # DMA tiles are 4096 cols so each fp16 partition-row is 8KB: the HWDGE
    # only splits a transfer across the 16 SDMA engines at >=8KB rows
    # (smaller rows aggregate into engine-atomic packets -> serial DMA).
    DTILE = 4096
    NDT = QP // DTILE              # 4 DMA tiles
    SUB = DTILE // TILE            # 2 psum sub-tiles per DMA tile

    with tile.TileContext(nc) as tc, ExitStack() as ctx:
        consts = ctx.enter_context(tc.tile_pool(name="consts", bufs=1))
        movh = ctx.enter_context(tc.tile_pool(name="movh", bufs=3))
        movl = ctx.enter_context(tc.tile_pool(name="movl", bufs=3))
        outp = ctx.enter_context(tc.tile_pool(name="outt", bufs=3))
        psump = ctx.enter_context(tc.tile_pool(name="ps", bufs=2, space="PSUM"))

        # weights go on the scalar HWDGE queue - the input queue (sync) must
        # start streaming xhi immediately
        whi_t = consts.tile([K, M], f16)
        nc.scalar.dma_start(whi_t[:], whi[:])
        wlo_t = consts.tile([K, M], f16)
        nc.scalar.dma_start(wlo_t[:], wlo[:])

        NJ = TILE // 512
        for it in range(NDT):
            sl_t = slice(it * DTILE, (it + 1) * DTILE)
            # two DMAs per operand: the HWDGE only spreads a transfer over
            # the 16 SDMA engines when partitions % 16 == 0 (or <= 16);
            # a single 109-partition DMA lands on ONE engine (16x slower)
            mh = movh.tile([K, DTILE], f16)
            nc.sync.dma_start(mh[0:KO, :], xhi[0:KO, sl_t])
            nc.sync.dma_start(mh[KO:K, :], xhi[KO:K, sl_t])
            ml = movl.tile([K, DTILE], f16)
            nc.sync.dma_start(ml[0:KO, :], xlo[0:KO, sl_t])
            nc.sync.dma_start(ml[KO:K, :], xlo[KO:K, sl_t])
            for s in range(SUB):
                i = it * SUB + s
                sl_i = slice(i * TILE, (i + 1) * TILE)
                ps = psump.tile([M, TILE], f32)
                # pass-major order: consecutive matmuls share the stationary
                # operand so (with ldw-opt) the weight reload is elided
                for j in range(NJ):
                    sl_p = slice(j * 512, (j + 1) * 512)
                    sl_j = slice(s * TILE + j * 512, s * TILE + (j + 1) * 512)
                    nc.tensor.matmul(ps[:, sl_p], whi_t[:], mh[:, sl_j],
                                     start=True, stop=False)
                for j in range(NJ):
                    sl_p = slice(j * 512, (j + 1) * 512)
                    sl_j = slice(s * TILE + j * 512, s * TILE + (j + 1) * 512)
                    nc.tensor.matmul(ps[:, sl_p], whi_t[:], ml[:, sl_j],
                                     start=False, stop=False)
                for j in range(NJ):
                    sl_p = slice(j * 512, (j + 1) * 512)
                    sl_j = slice(s * TILE + j * 512, s * TILE + (j + 1) * 512)
                    nc.tensor.matmul(ps[:, sl_p], wlo_t[:], mh[:, sl_j],
                                     start=False, stop=True)
                ot = outp.tile([M, TILE], f32)
                if i % 2 == 0:
                    nc.scalar.copy(ot[:], ps[:])
                else:
                    nc.vector.tensor_copy(ot[:], ps[:])
                nc.scalar.dma_start(out[:, sl_i], ot[:])

    nc.compile()
    return nc


def _get_program():
    if "nc" not in _STATE:
        _patch_ldw_opt()
        nc = _build_fp16x3() if MODE == "fp16x3" else _build_fp32()
        # The neuron compile cache keys on the HLO wrapper, which does NOT
        # include the embedded BIR - two different bass programs with the
        # same I/O signature would collide.  Key the cache dir on the BIR.
        import hashlib, tempfile
        digest = hashlib.sha256(nc.to_json_bytes()).hexdigest()[:16]
        os.environ["NEURON_COMPILE_CACHE_URL"] = os.path.join(
            tempfile.gettempdir(), f"neuron-cache-{digest}")
        _STATE["nc"] = nc
    return _STATE["nc"]


def _umeyama_params(Offset, Posmap_kpt, mean_posmap, uv_kpt):
    """Host-side: gather 68 keypoints, per-sample Umeyama in f64.
    Returns G[b,d,c] = revert[c] * sR[b,d,c] and t[b,d]."""
    h_idx = uv_kpt[:, 0].astype(np.int64)
    w_idx = uv_kpt[:, 1].astype(np.int64)
    # src[b,k,c] = revert[c] * (4*Offset[b,c,h,w] + mean[c,h,w])
    src = ((Offset[:, :, h_idx, w_idx] * OFFSET_SCALE
            + mean_posmap[:, h_idx, w_idx][None])
           * REVERT[None, :, None]).transpose(0, 2, 1).astype(np.float64)
    dst = Posmap_kpt[:, :, h_idx, w_idx].transpose(0, 2, 1).astype(np.float64)

    n = src.shape[1]
    mu_s = src.mean(axis=1)                      # [B,3]
    mu_d = dst.mean(axis=1)
    sd = src - mu_s[:, None, :]
    dd = dst - mu_d[:, None, :]
    A = np.einsum('bkd,bkc->bdc', dd, sd) / n    # [B,3,3] cross-covariance
    det = np.linalg.det(A)
    dvec = np.ones((B, 3))
    dvec[det < 0, 2] = -1.0
    U, S, Vt = np.linalg.svd(A)
    R = np.einsum('bik,bk,bkj->bij', U, dvec, Vt)
    var_s = (sd * sd).sum(axis=(1, 2)) / n
    scale = np.einsum('bk,bk->b', S, dvec) / var_s
    sR = scale[:, None, None] * R                # [B,3,3]
    t = mu_d - np.einsum('bij,bj->bi', sR, mu_s)
    G = sR * REVERT[None, None, :].astype(np.float64)   # fold revert into c
    return G, t


def _weights_for_core(G, t, core):
    """W[k, m]: m = g*24 + b*3 + d.  Rows: 96 Offset (block-diag per g,b),
    12 mean (block-diag per g, shared over b), 1 ones row (bias t)."""
    Wm = np.zeros((K, M), np.float32)
    Gc = G[core * BPC:(core + 1) * BPC]          # [8,3(d),3(c)]
    tc_ = t[core * BPC:(core + 1) * BPC]         # [8,3]
    for g in range(NG):
        for b in range(BPC):
            for c in range(C):
                for d in range(C):
                    m = g * 24 + b * 3 + d
                    Wm[g * 24 + b * 3 + c, m] = OFFSET_SCALE * Gc[b, d, c]
                    Wm[KO + g * 3 + c, m] = Gc[b, d, c]
                    Wm[KO + 12, m] = tc_[b, d]
    return Wm


def _split16(a):
    hi = a.astype(np.float16)
    lo = (a.astype(np.float32) - hi.astype(np.float32)).astype(np.float16)
    return np.ascontiguousarray(hi), np.ascontiguousarray(lo)


def kernel(Offset, Posmap_kpt, mean_posmap, uv_kpt):
    global LAST_RESULTS
    from concourse import bass_utils

    Offset = np.ascontiguousarray(np.asarray(Offset, dtype=np.float32))
    Posmap_kpt = np.asarray(Posmap_kpt, dtype=np.float32)
    mean_posmap = np.ascontiguousarray(np.asarray(mean_posmap, dtype=np.float32))
    uv = np.asarray(uv_kpt)

    G, t = _umeyama_params(Offset, Posmap_kpt, mean_posmap, uv)

    # [12, QP]: row g*3+c = mean[c, pixel-quarter g]; + ones row for bias
    meanr = np.ascontiguousarray(
        mean_posmap.reshape(C, NG, QP).transpose(1, 0, 2).reshape(NG * C, QP))
    meanr13 = np.concatenate([meanr, np.ones((1, QP), np.float32)], axis=0)

    in_maps = []
    for core in range(N_CORES):
        osh = Offset[core * BPC:(core + 1) * BPC]                 # [8,3,256,256]
        offs = np.ascontiguousarray(
            osh.reshape(BPC * C, NG, QP).transpose(1, 0, 2).reshape(KO, QP))
        Wm = _weights_for_core(G, t, core)
        if MODE == "fp16x3":
            x = np.concatenate([offs, meanr13], axis=0)           # [109, QP]
            xh, xl = _split16(x)
            wh, wl = _split16(Wm)
            in_maps.append({"xhi": xh, "xlo": xl, "whi": wh, "wlo": wl})
        else:
            in_maps.append({"offs": offs, "meanr": meanr13, "wmat": Wm})

    nc = _get_program()
    res = bass_utils.run_bass_kernel_spmd(nc, in_maps, core_ids=list(range(N_CORES)))
    LAST_RESULTS = res

    parts = []
    for core in range(N_CORES):
        o = res.results[core]["out"]                              # [96, QP]
        parts.append(o.reshape(NG, BPC * C, QP).transpose(1, 0, 2)
                      .reshape(BPC, C, H, W))
    return np.ascontiguousarray(np.concatenate(parts, axis=0)).astype(np.float32)


# revision 9
# speedup vs baseline: 5.6365x; 1.2361x over previous
"""Trainium2 Bass kernel for nn_EstimateRebuildModule.

Reference computation:
    offsetmap = (Offset*4 + mean_posmap) -> [B,H,W,3], y-channel sign-flipped
    src/dst   = 68 keypoints gathered from offsetmap / Posmap_kpt
    per-sample Umeyama similarity (3x3 SVD) -> sR, t
    out       = offsetmap @ sR.T + t -> [B,3,H,W]

Strategy (pure data parallel, batch sharded across 8 NeuronCores):
  * The per-sample transform only needs 68 pixels of each input to estimate
    sR/t; that tiny reduction (64 3x3 SVDs) runs on the host in f64.
  * The memory-bound bulk - reading Offset (48MB), applying the per-sample
    3x3 affine map to every pixel, writing the output (48MB) - runs on
    device.  Posmap_kpt's 48MB is never touched by the device (only its 68
    gathered pixels matter), so per-core traffic is ~12.9MB.
  * On each core the whole pixel transform is one stationary matmul:
    K=109 rows = 96 Offset rows (8 samples x 3 ch x 4 pixel-groups,
    block-diagonal weights) + 12 mean_posmap rows + 1 ones row (bias t),
    M=96 output rows, streamed over 16384 pixel columns.
  * MODE fp16x3: operands are split on the host into fp16 hi+lo halves
    (same DMA bytes as fp32) and the product is computed as
    Whi@xhi + Whi@xlo + Wlo@xhi, three 1-cycle/column fp16 matmuls -
    vs fp32's single 4-cycle/column pass.  absmax error matches fp32.
"""

import os
import numpy as np

B, C, H, W = 64, 3, 256, 256
N_CORES = 8
BPC = B // N_CORES          # samples per core
PIX = H * W                 # 65536
NG = 4                      # pixel groups packed into the partition dim
QP = PIX // NG              # 16384 pixel columns per group
KO = NG * BPC * C           # 96 Offset rows
KM = NG * C + 1             # 12 mean rows + 1 ones row
K = KO + KM                 # 109
M = NG * BPC * C            # 96 output rows
KP = 112                    # K padded to a multiple of 16 (DMA engine split)
TILE = 2048                 # pixel columns per tile
NTILES = QP // TILE         # 8
REVERT = np.array([1.0, -1.0, 1.0], np.float32)
OFFSET_SCALE = 4.0

MODE = os.environ.get("KERNEL_MODE", "fp16x3")   # "fp32" | "fp16x3"
LDW_OPT = os.environ.get("KERNEL_LDW_OPT", "1") == "1"

_STATE = {}
LAST_RESULTS = None


def _patch_ldw_opt():
    """Enable walrus's redundant-LDWEIGHTS elision (--enable-ldw-opt).
    concourse hardcodes it off; with a stationary weight reused across many
    matmuls the reloads are pure overhead (~190ns each on PE)."""
    if _STATE.get("ldw_patched") or not LDW_OPT:
        return
    from concourse import bass_utils
    orig = bass_utils.run_command

    def run_command_ldw(argv, **kw):
        argv = ["--enable-ldw-opt=true" if a == "--enable-ldw-opt=false" else a
                for a in argv]
        return orig(argv, **kw)

    bass_utils.run_command = run_command_ldw
    _STATE["ldw_patched"] = True


def _build_fp32():
    import concourse.bacc as bacc
    import concourse.tile as tile
    from concourse import mybir
    from contextlib import ExitStack

    f32 = mybir.dt.float32
    nc = bacc.Bacc("TRN2", target_bir_lowering=False, debug=False,
                   num_devices=N_CORES)
    offs = nc.dram_tensor("offs", [KO, QP], f32, kind="ExternalInput").ap()
    meanr = nc.dram_tensor("meanr", [KM, QP], f32, kind="ExternalInput").ap()
    wmat = nc.dram_tensor("wmat", [K, M], f32, kind="ExternalInput").ap()
    out = nc.dram_tensor("out", [M, QP], f32, kind="ExternalOutput").ap()

    with tile.TileContext(nc) as tc, ExitStack() as ctx:
        consts = ctx.enter_context(tc.tile_pool(name="consts", bufs=1))
        movp = ctx.enter_context(tc.tile_pool(name="mov", bufs=3))
        outp = ctx.enter_context(tc.tile_pool(name="outt", bufs=3))
        psump = ctx.enter_context(tc.tile_pool(name="ps", bufs=2, space="PSUM"))

        wt = consts.tile([K, M], f32)
        nc.sync.dma_start(wt[:], wmat[:])

        for i in range(NTILES):
            sl_i = slice(i * TILE, (i + 1) * TILE)
            mov = movp.tile([K, TILE], f32)
            nc.sync.dma_start(mov[0:KO, :], offs[:, sl_i])
            nc.sync.dma_start(mov[KO:K, :], meanr[:, sl_i])
            ps = psump.tile([M, TILE], f32)
            for j in range(TILE // 512):
                sl_j = slice(j * 512, (j + 1) * 512)
                nc.tensor.matmul(ps[:, sl_j], wt[:], mov[:, sl_j],
                                 start=True, stop=True)
            ot = outp.tile([M, TILE], f32)
            if i % 2 == 0:
                nc.scalar.copy(ot[:], ps[:])
            else:
                nc.vector.tensor_copy(ot[:], ps[:])
            nc.scalar.dma_start(out[:, sl_i], ot[:])

    nc.compile()
    return nc


def _build_fp16x3():
    import concourse.bacc as bacc
    import concourse.tile as tile
    from concourse import mybir
    from contextlib import ExitStack

    f32 = mybir.dt.float32
    f16 = mybir.dt.float16
    nc = bacc.Bacc("TRN2", target_bir_lowering=False, debug=False,
                   num_devices=N_CORES)
    xhi = nc.dram_tensor("xhi", [KP, QP], f16, kind="ExternalInput").ap()
    xlo = nc.dram_tensor("xlo", [KP, QP], f16, kind="ExternalInput").ap()
    whi = nc.dram_tensor("whi", [KP, M], f16, kind="ExternalInput").ap()
    wlo = nc.dram_tensor("wlo", [KP, M], f16, kind="ExternalInput").ap()
    out = nc.dram_tensor("out", [M, QP], f32, kind="ExternalOutput").ap()

    # DMA tiles are 4096 cols so each fp16 partition-row is 8KB, and all
    # partition counts are multiples of 16: the HWDGE only spreads a
    # transfer over the 16 SDMA engines under those conditions (otherwise
    # the whole transfer lands on ONE engine, 16x slower).
    DTILE = 4096
    NDT = QP // DTILE              # 4 DMA tiles
    SUB = DTILE // TILE            # 2 psum sub-tiles per DMA tile

    with tile.TileContext(nc) as tc, ExitStack() as ctx:
        consts = ctx.enter_context(tc.tile_pool(name="consts", bufs=1))
        movh = ctx.enter_context(tc.tile_pool(name="movh", bufs=4))
        movl = ctx.enter_context(tc.tile_pool(name="movl", bufs=4))
        outp = ctx.enter_context(tc.tile_pool(name="outt", bufs=4))
        psump = ctx.enter_context(tc.tile_pool(name="ps", bufs=2, space="PSUM"))

        # weights go on the scalar HWDGE queue - the input queue (sync) must
        # start streaming xhi immediately
        whi_t = consts.tile([KP, M], f16)
        nc.scalar.dma_start(whi_t[:], whi[:])
        wlo_t = consts.tile([KP, M], f16)
        nc.scalar.dma_start(wlo_t[:], wlo[:])

        NJ = TILE // 512
        for it in range(NDT):
            sl_t = slice(it * DTILE, (it + 1) * DTILE)
            mh = movh.tile([KP, DTILE], f16)
            nc.sync.dma_start(mh[:], xhi[:, sl_t])
            ml = movl.tile([KP, DTILE], f16)
            nc.sync.dma_start(ml[:], xlo[:, sl_t])
            for s in range(SUB):
                i = it * SUB + s
                sl_i = slice(i * TILE, (i + 1) * TILE)
                ps = psump.tile([M, TILE], f32)
                # pass-major order: consecutive matmuls share the stationary
                # operand; the PE pipelines back-to-back same-weight matmuls
                for j in range(NJ):
                    sl_p = slice(j * 512, (j + 1) * 512)
                    sl_j = slice(s * TILE + j * 512, s * TILE + (j + 1) * 512)
                    nc.tensor.matmul(ps[:, sl_p], whi_t[0:K, :], mh[0:K, sl_j],
                                     start=True, stop=False)
                for j in range(NJ):
                    sl_p = slice(j * 512, (j + 1) * 512)
                    sl_j = slice(s * TILE + j * 512, s * TILE + (j + 1) * 512)
                    nc.tensor.matmul(ps[:, sl_p], whi_t[0:K, :], ml[0:K, sl_j],
                                     start=False, stop=False)
                for j in range(NJ):
                    sl_p = slice(j * 512, (j + 1) * 512)
                    sl_j = slice(s * TILE + j * 512, s * TILE + (j + 1) * 512)
                    nc.tensor.matmul(ps[:, sl_p], wlo_t[0:K, :], mh[0:K, sl_j],
                                     start=False, stop=True)
                ot = outp.tile([M, TILE], f32)
                if i % 2 == 0:
                    nc.scalar.copy(ot[:], ps[:])
                else:
                    nc.vector.tensor_copy(ot[:], ps[:])
                nc.scalar.dma_start(out[:, sl_i], ot[:])

    nc.compile()
    return nc


def _get_program():
    if "nc" not in _STATE:
        _patch_ldw_opt()
        nc = _build_fp16x3() if MODE == "fp16x3" else _build_fp32()
        # The neuron compile cache keys on the HLO wrapper, which does NOT
        # include the embedded BIR - two different bass programs with the
        # same I/O signature would collide.  Key the cache dir on the BIR.
        import hashlib, tempfile
        digest = hashlib.sha256(nc.to_json_bytes()).hexdigest()[:16]
        os.environ["NEURON_COMPILE_CACHE_URL"] = os.path.join(
            tempfile.gettempdir(), f"neuron-cache-{digest}")
        _STATE["nc"] = nc
    return _STATE["nc"]


def _umeyama_params(Offset, Posmap_kpt, mean_posmap, uv_kpt):
    """Host-side: gather 68 keypoints, per-sample Umeyama in f64.
    Returns G[b,d,c] = revert[c] * sR[b,d,c] and t[b,d]."""
    h_idx = uv_kpt[:, 0].astype(np.int64)
    w_idx = uv_kpt[:, 1].astype(np.int64)
    # src[b,k,c] = revert[c] * (4*Offset[b,c,h,w] + mean[c,h,w])
    src = ((Offset[:, :, h_idx, w_idx] * OFFSET_SCALE
            + mean_posmap[:, h_idx, w_idx][None])
           * REVERT[None, :, None]).transpose(0, 2, 1).astype(np.float64)
    dst = Posmap_kpt[:, :, h_idx, w_idx].transpose(0, 2, 1).astype(np.float64)

    n = src.shape[1]
    mu_s = src.mean(axis=1)                      # [B,3]
    mu_d = dst.mean(axis=1)
    sd = src - mu_s[:, None, :]
    dd = dst - mu_d[:, None, :]
    A = np.einsum('bkd,bkc->bdc', dd, sd) / n    # [B,3,3] cross-covariance
    det = np.linalg.det(A)
    dvec = np.ones((B, 3))
    dvec[det < 0, 2] = -1.0
    U, S, Vt = np.linalg.svd(A)
    R = np.einsum('bik,bk,bkj->bij', U, dvec, Vt)
    var_s = (sd * sd).sum(axis=(1, 2)) / n
    scale = np.einsum('bk,bk->b', S, dvec) / var_s
    sR = scale[:, None, None] * R                # [B,3,3]
    t = mu_d - np.einsum('bij,bj->bi', sR, mu_s)
    G = sR * REVERT[None, None, :].astype(np.float64)   # fold revert into c
    return G, t


def _weights_for_core(G, t, core):
    """W[k, m]: m = g*24 + b*3 + d.  Rows: 96 Offset (block-diag per g,b),
    12 mean (block-diag per g, shared over b), 1 ones row (bias t)."""
    Wm = np.zeros((K, M), np.float32)
    Gc = G[core * BPC:(core + 1) * BPC]          # [8,3(d),3(c)]
    tc_ = t[core * BPC:(core + 1) * BPC]         # [8,3]
    for g in range(NG):
        for b in range(BPC):
            for c in range(C):
                for d in range(C):
                    m = g * 24 + b * 3 + d
                    Wm[g * 24 + b * 3 + c, m] = OFFSET_SCALE * Gc[b, d, c]
                    Wm[KO + g * 3 + c, m] = Gc[b, d, c]
                    Wm[KO + 12, m] = tc_[b, d]
    return Wm


def _split16(a):
    hi = a.astype(np.float16)
    lo = (a.astype(np.float32) - hi.astype(np.float32)).astype(np.float16)
    return np.ascontiguousarray(hi), np.ascontiguousarray(lo)


def kernel(Offset, Posmap_kpt, mean_posmap, uv_kpt):
    global LAST_RESULTS
    from concourse import bass_utils

    Offset = np.ascontiguousarray(np.asarray(Offset, dtype=np.float32))
    Posmap_kpt = np.asarray(Posmap_kpt, dtype=np.float32)
    mean_posmap = np.ascontiguousarray(np.asarray(mean_posmap, dtype=np.float32))
    uv = np.asarray(uv_kpt)

    G, t = _umeyama_params(Offset, Posmap_kpt, mean_posmap, uv)

    # [12, QP]: row g*3+c = mean[c, pixel-quarter g]; + ones row for bias
    meanr = np.ascontiguousarray(
        mean_posmap.reshape(C, NG, QP).transpose(1, 0, 2).reshape(NG * C, QP))
    meanr13 = np.concatenate([meanr, np.ones((1, QP), np.float32)], axis=0)

    in_maps = []
    for core in range(N_CORES):
        osh = Offset[core * BPC:(core + 1) * BPC]                 # [8,3,256,256]
        offs = np.ascontiguousarray(
            osh.reshape(BPC * C, NG, QP).transpose(1, 0, 2).reshape(KO, QP))
        Wm = _weights_for_core(G, t, core)
        if MODE == "fp16x3":
            x = np.concatenate(
                [offs, meanr13, np.zeros((KP - K, QP), np.float32)], axis=0)
            xh, xl = _split16(x)                                  # [112, QP]
            Wp = np.concatenate([Wm, np.zeros((KP - K, M), np.float32)], axis=0)
            wh, wl = _split16(Wp)
            in_maps.append({"xhi": xh, "xlo": xl, "whi": wh, "wlo": wl})
        else:
            in_maps.append({"offs": offs, "meanr": meanr13, "wmat": Wm})

    nc = _get_program()
    res = bass_utils.run_bass_kernel_spmd(nc, in_maps, core_ids=list(range(N_CORES)))
    LAST_RESULTS = res

    parts = []
    for core in range(N_CORES):
        o = res.results[core]["out"]                              # [96, QP]
        parts.append(o.reshape(NG, BPC * C, QP).transpose(1, 0, 2)
                      .reshape(BPC, C, H, W))
    return np.ascontiguousarray(np.concatenate(parts, axis=0)).astype(np.float32)
